# revision 1
# baseline (speedup 1.0000x reference)
"""Weighted-BCE + masked-MSE loss on 8 Trainium2 cores (pure data parallel).

Math (t in {0,1} exactly):
  class_sum = sum(bce * w)
            = -(w1 * sum(t*ln p) + w0 * (sum(ln(1-p)) - sum(t*ln(1-p))))
  masked sq = (1-t)*(ro-rt)^2  summed as  sum(dd^2) - sum(t*dd^2)
  cnt_zeros = N - sum(t)
Each core reduces its shard to 6 scalars; host combines and applies weights.

Engine mix per tile (balanced so DMA is the bottleneck):
  ACT : l1=Ln(p); l0=Ln(1-p)[+accum Sl0]; sq=Square(dd)[+accum Sdd2]
  DVE : three fused product+accum STT ops: t*l1, t*l0, t*sq
  Pool: dd=ro-rt (the only 2-input op); count = copy(t)+accum
"""

import os
import sys

for _p in ("/opt/trn_rl_repo", "/root/.axon_site/_ro/trn_rl_repo"):
    if os.path.isdir(_p) and _p not in sys.path:
        sys.path.insert(0, _p)

import numpy as np

import concourse.bacc as bacc
import concourse.bass_isa as bass_isa
import concourse.mybir as mybir
from concourse import tile
from concourse.bass_utils import run_bass_kernel_spmd

N = 16777216
NCORES = 8
NSHARD = N // NCORES  # 2097152
P = 128
F = 1024
NT = NSHARD // (P * F)  # 16

_F32 = mybir.dt.float32

LAST_RESULTS = None  # test harness peeks at exec_time_ns / trace path


def _build_nc():
    AF = mybir.ActivationFunctionType
    OP = mybir.AluOpType
    AX = mybir.AxisListType

    nc = bacc.Bacc(
        "TRN2", target_bir_lowering=False, debug=False, num_devices=NCORES
    )
    p_d = nc.dram_tensor("p", [NT, P, F], _F32, kind="ExternalInput")
    t_d = nc.dram_tensor("t", [NT, P, F], _F32, kind="ExternalInput")
    ro_d = nc.dram_tensor("ro", [NT, P, F], _F32, kind="ExternalInput")
    rt_d = nc.dram_tensor("rt", [NT, P, F], _F32, kind="ExternalInput")
    out_d = nc.dram_tensor("out", [1, 6], _F32, kind="ExternalOutput")

    with tile.TileContext(nc) as tc:
        with (
            tc.tile_pool(name="io", bufs=4) as io,
            tc.tile_pool(name="work", bufs=2) as work,
            tc.tile_pool(name="junkp", bufs=1) as junkp,
            tc.tile_pool(name="stats", bufs=1) as stats,
            tc.tile_pool(name="psum", bufs=1, space="PSUM") as psum,
        ):
            acc_tl1 = stats.tile([P, NT], _F32)  # sum t*ln(p) per tile col
            acc_tl0 = stats.tile([P, NT], _F32)  # sum t*ln(1-p)
            acc_l0 = stats.tile([P, NT], _F32)  # sum ln(1-p)
            acc_sq = stats.tile([P, NT], _F32)  # sum (ro-rt)^2
            acc_tsq = stats.tile([P, NT], _F32)  # sum t*(ro-rt)^2

            # count = sum(t) runs on the otherwise-idle PE:
            # ones[128,1].T @ t_chunk[128,512] accumulated into one PSUM bank
            ones = stats.tile([P, 1], _F32)
            nc.vector.memset(ones[:], 1.0)
            psum_cnt = psum.tile([1, 512], _F32)
            NCHUNK = F // 512

            for i in range(NT):
                tp = io.tile([P, F], _F32, tag="p")
                tt = io.tile([P, F], _F32, tag="t")
                tro = io.tile([P, F], _F32, tag="ro")
                trt = io.tile([P, F], _F32, tag="rt")
                nc.sync.dma_start(tp[:], p_d[i, :, :])
                nc.sync.dma_start(tt[:], t_d[i, :, :])
                nc.sync.dma_start(tro[:], ro_d[i, :, :])
                nc.sync.dma_start(trt[:], rt_d[i, :, :])

                # Pool: dd = ro - rt (its one 2-input op)
                dd = work.tile([P, F], _F32, tag="dd")
                nc.gpsimd.tensor_sub(dd[:], tro[:], trt[:])

                # ACT: logs + square; accum_out reduces for free
                l1 = work.tile([P, F], _F32, tag="l1")
                nc.scalar.activation(l1[:], tp[:], AF.Ln)
                l0 = work.tile([P, F], _F32, tag="l0")
                nc.scalar.activation(
                    l0[:], tp[:], AF.Ln, bias=1.0, scale=-1.0,
                    accum_out=acc_l0[:, i : i + 1],
                )
                sq = work.tile([P, F], _F32, tag="sq")
                nc.scalar.activation(
                    sq[:], dd[:], AF.Square, accum_out=acc_sq[:, i : i + 1]
                )

                # DVE: fused product+accumulate dots (out is a [P,1]
                # broadcast write; only accum_out matters)
                junk = junkp.tile([P, 1], _F32, tag="junk")
                nc.vector.scalar_tensor_tensor(
                    junk[:].broadcast_to([P, F]), tt[:], 1.0, l1[:],
                    OP.mult, OP.mult, accum_out=acc_tl1[:, i : i + 1],
                )
                junk2 = junkp.tile([P, 1], _F32, tag="junk2")
                nc.vector.scalar_tensor_tensor(
                    junk2[:].broadcast_to([P, F]), tt[:], 1.0, l0[:],
                    OP.mult, OP.mult, accum_out=acc_tl0[:, i : i + 1],
                )
                junk3 = junkp.tile([P, 1], _F32, tag="junk3")
                nc.vector.scalar_tensor_tensor(
                    junk3[:].broadcast_to([P, F]), tt[:], 1.0, sq[:],
                    OP.mult, OP.mult, accum_out=acc_tsq[:, i : i + 1],
                )

                # PE: accumulate column-sums of t into psum_cnt
                for c in range(NCHUNK):
                    nc.tensor.matmul(
                        psum_cnt[0:1, :],
                        ones[:, 0:1],
                        tt[:, c * 512 : (c + 1) * 512],
                        start=(i == 0 and c == 0),
                        stop=(i == NT - 1 and c == NCHUNK - 1),
                    )

            # Fold per-tile partials into out[1,6]
            red = stats.tile([P, 8], _F32)
            for j, acc in enumerate((acc_tl1, acc_tl0, acc_l0, acc_sq, acc_tsq)):
                nc.vector.tensor_reduce(red[:, j : j + 1], acc[:], AX.X, OP.add)
            tot = stats.tile([P, 8], _F32)
            nc.gpsimd.partition_all_reduce(
                tot[:, 0:5], red[:, 0:5], 128, bass_isa.ReduceOp.add
            )
            nc.vector.tensor_reduce(tot[0:1, 5:6], psum_cnt[0:1, :], AX.X, OP.add)
            nc.sync.dma_start(out_d[:], tot[0:1, 0:6])

    # Bacc pipeline: splits multi-wait sync (TRN2 allows 1 wait/inst),
    # lowers extended-ISA .instr bytes, register allocation, etc.
    nc.compile()
    return nc


def kernel(class_output, reg_output, class_target, reg_target, class_weights):
    global LAST_RESULTS
    nc = _build_nc()

    def shards(a):
        a = np.ascontiguousarray(np.asarray(a, dtype=np.float32))
        return [
            a[c * NSHARD : (c + 1) * NSHARD].reshape(NT, P, F) for c in range(NCORES)
        ]

    ps = shards(class_output)
    ts = shards(class_target)
    ros = shards(reg_output)
    rts = shards(reg_target)
    in_maps = [
        {"p": ps[c], "t": ts[c], "ro": ros[c], "rt": rts[c]} for c in range(NCORES)
    ]

    res = run_bass_kernel_spmd(nc, in_maps, core_ids=list(range(NCORES)))
    LAST_RESULTS = res

    parts = np.stack([np.asarray(res.results[c]["out"][0]) for c in range(NCORES)])
    tot = parts.sum(axis=0, dtype=np.float64)
    s_tl1, s_tl0, s_l0, s_sq, s_tsq, s_t = tot

    w0 = float(np.asarray(class_weights)[0, 0])
    w1 = float(np.asarray(class_weights)[0, 1])
    class_loss = -(w1 * s_tl1 + w0 * (s_l0 - s_tl0)) / N
    cnt = N - s_t
    reg_loss = ((s_sq - s_tsq) / cnt) if cnt > 0 else 0.0
    return np.float32(0.5 * class_loss + 0.5 * reg_loss)

